# revision 1
# baseline (speedup 1.0000x reference)
"""BinaryWeightConv2d on Trainium2 — 8-core data-parallel over batch.

Reference computation (fp32):
    scale = clip(mean|w| over (in,kh,kw), 1e-8)          # per out-channel
    bw    = sign(w) * scale
    out   = conv2d(x, bw, stride 1, pad 1) + bias
    y     = ternary(out): 1 if out > 0.5, -1 if out < -0.5, else 0

Kernel strategy:
  - Shard the batch (32) over 8 cores, 4 images each; replicate the tiny
    binarized weights (per the data-parallel sharding hint).
  - Host side: binarize weights to +-1 sign matrices; fold scale & bias into
    per-output-channel thresholds  hi = (0.5-b)/s,  lo = (-0.5-b)/s, so the
    device only computes the +-1 convolution and two compares.
  - Device: conv = 9 shifted-window matmuls (3x3 taps) accumulating in PSUM;
    contraction over C=128 = the partition dim.  x is host-padded to 58x58
    per image so every tap window is one contiguous SBUF slice.
  - Matmul dtype: fp16 hi/lo pair (x = x_h + x_l, both fp16; +-1 weights are
    exact in fp16) -> 18 accumulating matmuls per PSUM tile.  Result matches
    fp32 accumulation to ~1e-7 relative (measured on HW), giving a final
    ternary relative error ~5.7e-4 (the intrinsic fp32 reordering noise
    level).  A float32r variant ("f32r") is ~1.5x faster on 8 cores but has
    ~9e-3 ternary relative error (reduced-precision PE multiplies).
  - Epilogue per PSUM tile, 2 vector ops:  b = (raw < lo);
    y = (raw > hi) - b   in {-1, 0, 1}.
  - Outputs are stored as full padded [C, 464] tiles (contiguous DMA
    segments); the host strips the 2 junk columns per 58-wide row.
"""

import os
import numpy as np

N, C, H, W = 32, 128, 56, 56
O = 256
NCORES = 8
NPC = N // NCORES           # images per core
HP, WP = H + 2, W + 2       # padded spatial
IMG = HP * WP               # 3364
XCOLS = NPC * IMG           # 13456
XCOLS_PAD = XCOLS + 64      # slack: the last tap of the last tile overreads 1
RB = 8                      # output rows per PSUM tile
NT = RB * WP                # 464 = PSUM tile free size (<= 512 bank limit)
NBLK = H // RB              # 7 row blocks
TAPS = [(kh, kw) for kh in range(3) for kw in range(3)]

MODE = os.environ.get("BWC_MODE", "f16p+j+ys")

_prog_cache = {}


def _build(mode, repeat=1):
    import concourse.tile as tile
    from concourse import mybir, bacc
    from contextlib import ExitStack

    dt = mybir.dt
    nc = bacc.Bacc()

    parts = mode.split("+")
    base, flags = parts[0], set(parts[1:])
    jpad = "j" in flags
    ydt_bf = "h" in flags       # store ternary output as bf16 (host converts)
    dmaless = "dl" in flags     # timing probe: ~zero output DMA volume
    act_out = "a" in flags      # issue output stores on the ACT HWDGE ring
    obufs = 16 if "o16" in flags else 6
    ysplit = "ys" in flags      # dedicated deep pool for DMA-held y tiles

    if base == "f16p":
        mm_dt, np_mm = dt.float16, np.float16
    elif base == "f32r":
        mm_dt, np_mm = dt.float32r, np.float32
    elif base == "f32":
        mm_dt, np_mm = dt.float32, np.float32
    else:
        raise ValueError(mode)
    pair = base == "f16p"

    xh_d = nc.declare_dram_parameter("xh", [C, XCOLS_PAD], mm_dt, isOutput=False)
    xl_d = (nc.declare_dram_parameter("xl", [C, XCOLS_PAD], mm_dt, isOutput=False)
            if pair else None)
    sw_d = nc.declare_dram_parameter("sw", [C, 9 * O], mm_dt, isOutput=False)
    thr_d = nc.declare_dram_parameter("thr", [C, 4], dt.float32, isOutput=False)
    out_dt = dt.bfloat16 if ydt_bf else dt.float32
    if jpad:
        out_d = nc.declare_dram_parameter("out", [NPC, 2, NBLK, C, NT],
                                          out_dt, isOutput=True)
    else:
        out_d = nc.declare_dram_parameter("out", [2, C, NPC, H, W],
                                          out_dt, isOutput=True)

    with tile.TileContext(nc) as tc, ExitStack() as ctx:
        inp = ctx.enter_context(tc.tile_pool(name="inp", bufs=2))
        outp = ctx.enter_context(tc.tile_pool(name="outp", bufs=4 if ysplit else obufs))
        ypool = (ctx.enter_context(tc.tile_pool(name="ypool", bufs=24))
                 if ysplit else outp)
        psum = ctx.enter_context(tc.tile_pool(name="psum", bufs=8, space="PSUM"))

        def body():
            t_w = inp.tile([C, 9 * O], mm_dt, tag="w")
            nc.sync.dma_start(t_w[:], sw_d[:])
            t_thr = inp.tile([C, 4], dt.float32, tag="thr")
            nc.sync.dma_start(t_thr[:], thr_d[:])

            t_xh = inp.tile([C, XCOLS_PAD], mm_dt, tag="xh")
            t_xl = (inp.tile([C, XCOLS_PAD], mm_dt, tag="xl", name="t_xl")
                    if pair else None)
            # chunked x loads (per image) so compute starts after chunk 0
            bounds = [0, IMG, 2 * IMG, 3 * IMG, XCOLS_PAD]
            for i in range(4):
                lo, hi = bounds[i], bounds[i + 1]
                nc.sync.dma_start(t_xh[:, lo:hi], xh_d[:, lo:hi])
                if pair:
                    nc.sync.dma_start(t_xl[:, lo:hi], xl_d[:, lo:hi])

            nmm = 18 if pair else 9
            for n in range(NPC):
                for oc in range(2):
                    hi_ap = t_thr[:, 2 * oc:2 * oc + 1]
                    lo_ap = t_thr[:, 2 * oc + 1:2 * oc + 2]
                    for j in range(NBLK):
                        h0 = j * RB
                        pt = psum.tile([C, 512], dt.float32, tag="pt")
                        pt = pt[:, :NT]
                        k = 0
                        for t, (kh, kw) in enumerate(TAPS):
                            base_off = n * IMG + (h0 + kh) * WP + kw
                            wt = t_w[:, t * O + oc * C: t * O + oc * C + C]
                            nc.tensor.matmul(pt, wt, t_xh[:, base_off:base_off + NT],
                                             start=(k == 0), stop=(k == nmm - 1))
                            k += 1
                            if pair:
                                nc.tensor.matmul(pt, wt, t_xl[:, base_off:base_off + NT],
                                                 start=False, stop=(k == nmm - 1))
                                k += 1
                        # ternary epilogue: y = (raw > hi) - (raw < lo)
                        b = outp.tile([C, NT], dt.float32, tag="b")
                        nc.vector.tensor_scalar(b[:], pt, lo_ap, None,
                                                mybir.AluOpType.is_lt)
                        y = ypool.tile([C, NT], out_dt, tag="y")
                        nc.vector.scalar_tensor_tensor(
                            y[:], pt, hi_ap, b[:],
                            mybir.AluOpType.is_gt, mybir.AluOpType.subtract)
                        if jpad:
                            out_eng = nc.scalar if act_out else nc.sync
                            if dmaless:
                                out_eng.dma_start(out_d[n, oc, j][:, :8], y[:, :8])
                            else:
                                out_eng.dma_start(out_d[n, oc, j], y[:])
                        else:
                            y_r = y[:].rearrange("p (r w) -> p r w", w=WP)[:, :, :W]
                            nc.sync.dma_start(out_d[oc, :, n, h0:h0 + RB, :], y_r)

        if repeat == 1:
            body()
        else:
            with tc.For_i(0, repeat, 1):
                body()

    nc.compile()
    return nc, np_mm


def _host_prep(x, weight, bias):
    scale = np.clip(np.mean(np.abs(weight), axis=(1, 2, 3)), 1e-8, None)  # [O]
    sw = np.sign(weight)                                                  # [O,C,3,3]
    hi = ((0.5 - bias.astype(np.float64)) / scale.astype(np.float64)).astype(np.float32)
    lo = ((-0.5 - bias.astype(np.float64)) / scale.astype(np.float64)).astype(np.float32)
    thr = np.stack([hi[:C], lo[:C], hi[C:], lo[C:]], axis=1).astype(np.float32)
    # lhsT layout: sw[c, t*O + o]
    swt = np.ascontiguousarray(sw.transpose(1, 2, 3, 0).reshape(C, 9 * O))
    # pad x to 58x58 and lay out [C, n*3364 + hp*58 + wp]
    xp = np.zeros((N, C, HP, WP), dtype=np.float32)
    xp[:, :, 1:-1, 1:-1] = x
    xp = xp.transpose(1, 0, 2, 3).reshape(C, N * IMG)
    return thr, swt, xp


def _make_in_maps(mode, thr, swt, xp):
    pair = mode.startswith("f16p")
    in_maps = []
    for c in range(NCORES):
        xc = np.zeros((C, XCOLS_PAD), dtype=np.float32)
        xc[:, :XCOLS] = xp[:, c * XCOLS:(c + 1) * XCOLS]
        m = {"thr": thr}
        if pair:
            xh = xc.astype(np.float16)
            m["xh"] = xh
            m["xl"] = (xc - xh.astype(np.float32)).astype(np.float16)
            m["sw"] = swt.astype(np.float16)
        else:
            m["xh"] = xc
            m["sw"] = swt.copy()
        in_maps.append(m)
    return in_maps


def kernel(x, weight, bias):
    from concourse.bass_utils import run_bass_kernel_spmd

    x = np.asarray(x, dtype=np.float32)
    weight = np.asarray(weight, dtype=np.float32)
    bias = np.asarray(bias, dtype=np.float32)

    thr, swt, xp = _host_prep(x, weight, bias)

    mode = MODE
    if mode not in _prog_cache:
        _prog_cache[mode] = _build(mode)
    nc, _ = _prog_cache[mode]

    in_maps = _make_in_maps(mode, thr, swt, xp)
    res = run_bass_kernel_spmd(nc, in_maps, list(range(NCORES)))

    # ---- gather per-core outputs -> [N, O, H, W] fp32 ----
    out = np.empty((N, O, H, W), dtype=np.float32)
    for c in range(NCORES):
        oc_out = res.results[c]["out"]
        if "+j" in mode:
            # [NPC, 2, NBLK, C, NT]: rows of 58, valid w < 56
            v = np.asarray(oc_out).astype(np.float32, copy=False)
            v = v.reshape(NPC, 2, NBLK, C, RB, WP)[:, :, :, :, :, :W]
            v = v.transpose(0, 1, 3, 2, 4, 5).reshape(NPC, O, H, W)
            out[c * NPC:(c + 1) * NPC] = v
        else:
            for oc in range(2):
                out[c * NPC:(c + 1) * NPC, oc * C:(oc + 1) * C] = \
                    oc_out[oc].transpose(1, 0, 2, 3)
    return out



# revision 4
# speedup vs baseline: 1.1157x; 1.1157x over previous
"""BinaryWeightConv2d on Trainium2 — 8-core data-parallel over batch.

Reference computation (fp32):
    scale = clip(mean|w| over (in,kh,kw), 1e-8)          # per out-channel
    bw    = sign(w) * scale
    out   = conv2d(x, bw, stride 1, pad 1) + bias
    y     = ternary(out): 1 if out > 0.5, -1 if out < -0.5, else 0

Kernel strategy:
  - Shard the batch (32) over 8 cores, 4 images each; replicate the tiny
    binarized weights (per the data-parallel sharding hint).
  - Host side: binarize weights to +-1 sign matrices; fold scale & bias into
    per-output-channel thresholds  hi = (0.5-b)/s,  lo = (-0.5-b)/s, so the
    device only computes the +-1 convolution and two compares.
  - Device: conv = 9 shifted-window matmuls (3x3 taps) accumulating in PSUM;
    contraction over C=128 = the partition dim.  x is host-padded to 58x58
    per image so every tap window is one contiguous SBUF slice.
  - Matmul dtype: fp16 hi/lo pair (x = x_h + x_l, both fp16; +-1 weights are
    exact in fp16) -> 18 accumulating matmuls per PSUM tile.  Result matches
    fp32 accumulation to ~1e-7 relative (measured on HW), giving a final
    ternary relative error ~5.7e-4 (the intrinsic fp32 reordering noise
    level).  A float32r variant ("f32r") is ~1.5x faster on 8 cores but has
    ~9e-3 ternary relative error (reduced-precision PE multiplies).
  - Epilogue per PSUM tile, 2 vector ops:  b = (raw < lo);
    y = (raw > hi) - b   in {-1, 0, 1}.
  - Outputs are stored as full padded [C, 464] tiles (contiguous DMA
    segments); the host strips the 2 junk columns per 58-wide row.
"""

import os
import numpy as np

N, C, H, W = 32, 128, 56, 56
O = 256
NCORES = 8
NPC = N // NCORES           # images per core
HP, WP = H + 2, W + 2       # padded spatial
IMG = HP * WP               # 3364
XCOLS = NPC * IMG           # 13456
XCOLS_PAD = XCOLS + 64      # slack: the last tap of the last tile overreads 1
RB = 8                      # output rows per PSUM tile
NT = RB * WP                # 464 = PSUM tile free size (<= 512 bank limit)
NBLK = H // RB              # 7 row blocks
TAPS = [(kh, kw) for kh in range(3) for kw in range(3)]

MODE = os.environ.get("BWC_MODE", "f16p+j+ys")

_prog_cache = {}


def _build(mode, repeat=1):
    import concourse.tile as tile
    from concourse import mybir, bacc
    from contextlib import ExitStack

    dt = mybir.dt
    nc = bacc.Bacc()

    parts = mode.split("+")
    base, flags = parts[0], set(parts[1:])
    jpad = "j" in flags
    ydt_bf = "h" in flags       # store ternary output as bf16 (host converts)
    dmaless = "dl" in flags     # timing probe: ~zero output DMA volume
    act_out = "a" in flags      # issue output stores on the ACT HWDGE ring
    obufs = 16 if "o16" in flags else 6
    ysplit = "ys" in flags      # dedicated deep pool for DMA-held y tiles

    if base == "f16p":
        mm_dt, np_mm = dt.float16, np.float16
    elif base == "f16s":
        mm_dt, np_mm = dt.float16, np.float16
    elif base == "bf16s":
        mm_dt, np_mm = dt.bfloat16, np.float32
    elif base == "f32r":
        mm_dt, np_mm = dt.float32r, np.float32
    elif base == "f32":
        mm_dt, np_mm = dt.float32, np.float32
    else:
        raise ValueError(mode)
    pair = base == "f16p"

    xh_d = nc.declare_dram_parameter("xh", [C, XCOLS_PAD], mm_dt, isOutput=False)
    xl_d = (nc.declare_dram_parameter("xl", [C, XCOLS_PAD], mm_dt, isOutput=False)
            if pair else None)
    sw_d = nc.declare_dram_parameter("sw", [C, 9 * O], mm_dt, isOutput=False)
    thr_d = nc.declare_dram_parameter("thr", [C, 4], dt.float32, isOutput=False)
    out_dt = dt.bfloat16 if ydt_bf else dt.float32
    if jpad:
        out_d = nc.declare_dram_parameter("out", [NPC, 2, NBLK, C, NT],
                                          out_dt, isOutput=True)
    else:
        out_d = nc.declare_dram_parameter("out", [2, C, NPC, H, W],
                                          out_dt, isOutput=True)

    with tile.TileContext(nc) as tc, ExitStack() as ctx:
        inp = ctx.enter_context(tc.tile_pool(name="inp", bufs=2))
        outp = ctx.enter_context(tc.tile_pool(name="outp", bufs=4 if ysplit else obufs))
        ypool = (ctx.enter_context(tc.tile_pool(name="ypool", bufs=24))
                 if ysplit else outp)
        psum = ctx.enter_context(tc.tile_pool(name="psum", bufs=8, space="PSUM"))

        def body():
            t_w = inp.tile([C, 9 * O], mm_dt, tag="w")
            nc.sync.dma_start(t_w[:], sw_d[:])
            t_thr = inp.tile([C, 4], dt.float32, tag="thr")
            nc.sync.dma_start(t_thr[:], thr_d[:])

            t_xh = inp.tile([C, XCOLS_PAD], mm_dt, tag="xh")
            t_xl = (inp.tile([C, XCOLS_PAD], mm_dt, tag="xl", name="t_xl")
                    if pair else None)
            # chunked x loads (per image) so compute starts after chunk 0
            bounds = [0, IMG, 2 * IMG, 3 * IMG, XCOLS_PAD]
            for i in range(4):
                lo, hi = bounds[i], bounds[i + 1]
                nc.sync.dma_start(t_xh[:, lo:hi], xh_d[:, lo:hi])
                if pair:
                    nc.sync.dma_start(t_xl[:, lo:hi], xl_d[:, lo:hi])

            nmm = 18 if pair else 9
            for n in range(NPC):
                for oc in range(2):
                    hi_ap = t_thr[:, 2 * oc:2 * oc + 1]
                    lo_ap = t_thr[:, 2 * oc + 1:2 * oc + 2]
                    for j in range(NBLK):
                        h0 = j * RB
                        pt = psum.tile([C, 512], dt.float32, tag="pt")
                        pt = pt[:, :NT]
                        k = 0
                        for t, (kh, kw) in enumerate(TAPS):
                            base_off = n * IMG + (h0 + kh) * WP + kw
                            wt = t_w[:, t * O + oc * C: t * O + oc * C + C]
                            nc.tensor.matmul(pt, wt, t_xh[:, base_off:base_off + NT],
                                             start=(k == 0), stop=(k == nmm - 1))
                            k += 1
                            if pair:
                                nc.tensor.matmul(pt, wt, t_xl[:, base_off:base_off + NT],
                                                 start=False, stop=(k == nmm - 1))
                                k += 1
                        # ternary epilogue: y = (raw > hi) - (raw < lo)
                        b = outp.tile([C, NT], dt.float32, tag="b")
                        nc.vector.tensor_scalar(b[:], pt, lo_ap, None,
                                                mybir.AluOpType.is_lt)
                        y = ypool.tile([C, NT], out_dt, tag="y")
                        nc.vector.scalar_tensor_tensor(
                            y[:], pt, hi_ap, b[:],
                            mybir.AluOpType.is_gt, mybir.AluOpType.subtract)
                        if jpad:
                            out_eng = nc.scalar if act_out else nc.sync
                            if dmaless:
                                out_eng.dma_start(out_d[n, oc, j][:, :8], y[:, :8])
                            else:
                                out_eng.dma_start(out_d[n, oc, j], y[:])
                        else:
                            y_r = y[:].rearrange("p (r w) -> p r w", w=WP)[:, :, :W]
                            nc.sync.dma_start(out_d[oc, :, n, h0:h0 + RB, :], y_r)

        if repeat == 1:
            body()
        else:
            with tc.For_i(0, repeat, 1):
                body()

    nc.compile()
    return nc, np_mm


def _host_prep(x, weight, bias):
    scale = np.clip(np.mean(np.abs(weight), axis=(1, 2, 3)), 1e-8, None)  # [O]
    sw = np.sign(weight)                                                  # [O,C,3,3]
    hi = ((0.5 - bias.astype(np.float64)) / scale.astype(np.float64)).astype(np.float32)
    lo = ((-0.5 - bias.astype(np.float64)) / scale.astype(np.float64)).astype(np.float32)
    thr = np.stack([hi[:C], lo[:C], hi[C:], lo[C:]], axis=1).astype(np.float32)
    # lhsT layout: sw[c, t*O + o]
    swt = np.ascontiguousarray(sw.transpose(1, 2, 3, 0).reshape(C, 9 * O))
    # pad x to 58x58 and lay out [C, n*3364 + hp*58 + wp]
    xp = np.zeros((N, C, HP, WP), dtype=np.float32)
    xp[:, :, 1:-1, 1:-1] = x
    xp = xp.transpose(1, 0, 2, 3).reshape(C, N * IMG)
    return thr, swt, xp


def _make_in_maps(mode, thr, swt, xp):
    base = mode.split("+")[0]
    pair = base == "f16p"
    in_maps = []
    for c in range(NCORES):
        xc = np.zeros((C, XCOLS_PAD), dtype=np.float32)
        xc[:, :XCOLS] = xp[:, c * XCOLS:(c + 1) * XCOLS]
        m = {"thr": thr}
        if pair:
            xh = xc.astype(np.float16)
            m["xh"] = xh
            m["xl"] = (xc - xh.astype(np.float32)).astype(np.float16)
            m["sw"] = swt.astype(np.float16)
        elif base == "f16s":
            m["xh"] = xc.astype(np.float16)
            m["sw"] = swt.astype(np.float16)
        elif base == "bf16s":
            import ml_dtypes
            m["xh"] = xc.astype(ml_dtypes.bfloat16)
            m["sw"] = swt.astype(ml_dtypes.bfloat16)
        else:
            m["xh"] = xc
            m["sw"] = swt.copy()
        in_maps.append(m)
    return in_maps


def kernel(x, weight, bias):
    from concourse.bass_utils import run_bass_kernel_spmd

    x = np.asarray(x, dtype=np.float32)
    weight = np.asarray(weight, dtype=np.float32)
    bias = np.asarray(bias, dtype=np.float32)

    thr, swt, xp = _host_prep(x, weight, bias)

    mode = MODE
    if mode not in _prog_cache:
        _prog_cache[mode] = _build(mode)
    nc, _ = _prog_cache[mode]

    in_maps = _make_in_maps(mode, thr, swt, xp)
    res = run_bass_kernel_spmd(nc, in_maps, list(range(NCORES)))

    # ---- gather per-core outputs -> [N, O, H, W] fp32 ----
    out = np.empty((N, O, H, W), dtype=np.float32)
    for c in range(NCORES):
        oc_out = res.results[c]["out"]
        if "+j" in mode:
            # [NPC, 2, NBLK, C, NT]: rows of 58, valid w < 56
            v = np.asarray(oc_out).astype(np.float32, copy=False)
            v = v.reshape(NPC, 2, NBLK, C, RB, WP)[:, :, :, :, :, :W]
            v = v.transpose(0, 1, 3, 2, 4, 5).reshape(NPC, O, H, W)
            out[c * NPC:(c + 1) * NPC] = v
        else:
            for oc in range(2):
                out[c * NPC:(c + 1) * NPC, oc * C:(oc + 1) * C] = \
                    oc_out[oc].transpose(1, 0, 2, 3)
    return out



# revision 33
# speedup vs baseline: 2.6810x; 2.4031x over previous
"""BinaryWeightConv2d on Trainium2 — 8-core data-parallel over batch.

Reference computation (fp32):
    scale = clip(mean|w| over (in,kh,kw), 1e-8)          # per out-channel
    bw    = sign(w) * scale
    out   = conv2d(x, bw, stride 1, pad 1) + bias
    y     = ternary(out): 1 if out > 0.5, -1 if out < -0.5, else 0

Kernel strategy (v3 family — current default):
  - Shard the batch (32) over 8 cores, 4 images each; replicate the tiny
    binarized weights (per the data-parallel sharding hint).
  - Host side: binarize weights to +-1 sign matrices; fold scale & bias into
    per-output-channel thresholds  hi = (0.5-b)/s,  lo = (-0.5-b)/s, so the
    device only computes the +-1 convolution and compares.
  - Device: conv = 9 shifted-window matmuls (3x3 taps) accumulating in PSUM;
    contraction over C=128 = the partition dim.  x is host-padded to 58x58
    per image; the matmul rhs is a strided [8 rows x 56 valid cols] window
    (448 columns — no junk columns on the PE).
  - Matmul dtype: fp16 single (x rounded to fp16; +-1 weights exact).
    Ternary rel err ~1.24e-2 on the fixed test inputs (gate: 2e-2).
  - Tap-major grouping: a 4-bank PSUM tile per group of 4 row-blocks,
    double-buffered (2 x 4 banks); each weight tap feeds 4 consecutive
    matmuls, amortizing LDWEIGHTS and PE queue overhead.
  - Group-fused epilogue, balanced across ACT and DVE per group parity:
      sgn groups:  a1 = Sign(raw - hi), a2 = Sign(raw - lo)   [2 ACT ops]
                   y = a1 + a2 in {-2, 0, 2}                  [1 DVE op]
      alt groups:  a1 = Sign(raw - hi)                        [1 ACT op]
                   b = (raw < lo); y = a1 - 2b in {-3,-1,1}   [2 DVE ops]
    Host decodes y*0.5 + parity*0.5 during the gather.  Epilogue ops span
    full 512-col PSUM banks (2D contiguous APs; 3D strided APs on ACT/DVE
    crash the HW) — junk columns are stripped on the host.
  - Output stored as bf16 [NPC, 2, C, NBLK*512]; stores issued on the ACT
    HWDGE ring so the SP ring only carries next-iteration input loads.
  - Timing builds unroll the body 2x inside the For_i hardware loop so tile
    pools ping-pong across iterations (fixed addresses inside a HW loop).
"""

import os
import numpy as np

N, C, H, W = 32, 128, 56, 56
O = 256
NCORES = 8
NPC = N // NCORES           # images per core
HP, WP = H + 2, W + 2       # padded spatial
IMG = HP * WP               # 3364
XCOLS = NPC * IMG           # 13456
XCOLS_PAD = XCOLS + 64      # slack: the last tap of the last tile overreads 1
RB = 8                      # output rows per PSUM tile
NT = RB * WP                # 464 = PSUM tile free size (<= 512 bank limit)
NBLK = H // RB              # 7 row blocks
TAPS = [(kh, kw) for kh in range(3) for kw in range(3)]

MODE = os.environ.get("BWC_MODE", "v3+fb+a")

_prog_cache = {}


def _parse_v2(mode):
    """v2 mode grammar: 'v2[+f16p|+f32r][+g<k>][+sgn][+v448][+f32out][+dl][+a]'"""
    parts = mode.split("+")
    assert parts[0] == "v2"
    flags = set(parts[1:])
    cfg = {
        "pair": "f16p" in flags,
        "f32r": "f32r" in flags,
        "G": 4,
        "sgn": "sgn" in flags,
        "v448": "v448" in flags,
        "ydt_bf": "f32out" not in flags,
        "dmaless": "dl" in flags,
        "act_out": "a" in flags,
        "t1": "t1" in flags,        # probe: single tap (wrong results)
        "mix": "mix" in flags,      # balance epilogue between ACT and DVE
    }
    for f in flags:
        if f.startswith("g") and f[1:].isdigit():
            cfg["G"] = int(f[1:])
    return cfg


def _build_v2(mode, repeat=1):
    import concourse.tile as tile
    from concourse import mybir, bacc
    from contextlib import ExitStack

    dt = mybir.dt
    nc = bacc.Bacc()
    cfg = _parse_v2(mode)
    G = cfg["G"]

    if cfg["f32r"]:
        mm_dt = dt.float32r
    else:
        mm_dt = dt.float16
    pair = cfg["pair"]
    sgn = cfg["sgn"]
    NTv = RB * W if cfg["v448"] else NT          # 448 or 464
    out_dt = dt.bfloat16 if cfg["ydt_bf"] else dt.float32

    xh_d = nc.declare_dram_parameter("xh", [C, XCOLS_PAD], mm_dt, isOutput=False)
    xl_d = (nc.declare_dram_parameter("xl", [C, XCOLS_PAD], mm_dt, isOutput=False)
            if pair else None)
    sw_d = nc.declare_dram_parameter("sw", [C, 9 * O], mm_dt, isOutput=False)
    thr_d = nc.declare_dram_parameter("thr", [C, 8], dt.float32, isOutput=False)
    out_d = nc.declare_dram_parameter("out", [NPC, 2, NBLK, C, NTv],
                                      out_dt, isOutput=True)

    taps = list(enumerate(TAPS))[:1] if cfg["t1"] else list(enumerate(TAPS))
    nmm = (2 if pair else 1) * len(taps)

    with tile.TileContext(nc) as tc, ExitStack() as ctx:
        inp = ctx.enter_context(tc.tile_pool(name="inp", bufs=2))
        outp = ctx.enter_context(tc.tile_pool(name="outp", bufs=8))
        ypool = ctx.enter_context(tc.tile_pool(name="ypool", bufs=24))
        psum = ctx.enter_context(tc.tile_pool(name="psum", bufs=8, space="PSUM"))

        # j-block groups for tap-major accumulation
        groups = [list(range(g, min(g + G, NBLK))) for g in range(0, NBLK, G)]

        def rhs_ap(t_x, n, j, kh, kw):
            base = n * IMG + (j * RB + kh) * WP + kw
            if cfg["v448"]:
                return (t_x[:, base:base + NT]
                        .rearrange("p (r w) -> p r w", w=WP)[:, :, :W])
            return t_x[:, base:base + NT]

        def body():
            t_w = inp.tile([C, 9 * O], mm_dt, tag="w")
            nc.sync.dma_start(t_w[:], sw_d[:])
            t_thr = inp.tile([C, 8], dt.float32, tag="thr")
            nc.sync.dma_start(t_thr[:], thr_d[:])

            t_xh = inp.tile([C, XCOLS_PAD], mm_dt, tag="xh")
            t_xl = (inp.tile([C, XCOLS_PAD], mm_dt, tag="xl", name="t_xl")
                    if pair else None)
            bounds = [0, IMG, 2 * IMG, 3 * IMG, XCOLS_PAD]
            for i in range(4):
                lo, hi = bounds[i], bounds[i + 1]
                nc.sync.dma_start(t_xh[:, lo:hi], xh_d[:, lo:hi])
                if pair:
                    nc.sync.dma_start(t_xl[:, lo:hi], xl_d[:, lo:hi])

            for n in range(NPC):
                for oc in range(2):
                    hi_ap = t_thr[:, 2 * oc:2 * oc + 1]
                    lo_ap = t_thr[:, 2 * oc + 1:2 * oc + 2]
                    nhi_ap = t_thr[:, 4 + 2 * oc:4 + 2 * oc + 1]
                    nlo_ap = t_thr[:, 4 + 2 * oc + 1:4 + 2 * oc + 2]
                    for grp in groups:
                        pts = {}
                        for j in grp:
                            pt = psum.tile([C, 512], dt.float32, tag="pt")
                            pts[j] = pt[:, :NTv]
                        # tap-major: one weight slice feeds all tiles in grp
                        k = 0
                        for t, (kh, kw) in taps:
                            wt = t_w[:, t * O + oc * C: t * O + oc * C + C]
                            for j in grp:
                                nc.tensor.matmul(
                                    pts[j], wt, rhs_ap(t_xh, n, j, kh, kw),
                                    start=(k == 0), stop=(k == nmm - 1))
                            if pair:
                                for j in grp:
                                    nc.tensor.matmul(
                                        pts[j], wt, rhs_ap(t_xl, n, j, kh, kw),
                                        start=False, stop=(k + 1 == nmm - 1))
                            k += 2 if pair else 1
                        for j in grp:
                            pt = pts[j]
                            y = ypool.tile([C, NTv], out_dt, tag="y")
                            if cfg["mix"]:
                                par = (j + oc * NBLK + n * 2 * NBLK) % 2
                                a1 = outp.tile([C, NTv], out_dt, tag="a1")
                                nc.scalar.activation(
                                    a1[:], pt, mybir.ActivationFunctionType.Sign,
                                    bias=nhi_ap)
                                if par == 0:
                                    a2 = outp.tile([C, NTv], out_dt, tag="a2")
                                    nc.scalar.activation(
                                        a2[:], pt,
                                        mybir.ActivationFunctionType.Sign,
                                        bias=nlo_ap)
                                    nc.vector.tensor_tensor(
                                        y[:], a1[:], a2[:], mybir.AluOpType.add)
                                else:
                                    b = outp.tile([C, NTv], out_dt, tag="b")
                                    nc.vector.tensor_scalar(
                                        b[:], pt, lo_ap, None,
                                        mybir.AluOpType.is_lt)
                                    nc.vector.scalar_tensor_tensor(
                                        y[:], b[:], -2.0, a1[:],
                                        mybir.AluOpType.mult, mybir.AluOpType.add)
                            elif sgn:
                                # y_dev = sign(raw-hi) + sign(raw-lo); host *0.5
                                a1 = outp.tile([C, NTv], out_dt, tag="a1")
                                nc.scalar.activation(
                                    a1[:], pt, mybir.ActivationFunctionType.Sign,
                                    bias=nhi_ap)
                                a2 = outp.tile([C, NTv], out_dt, tag="a2")
                                nc.scalar.activation(
                                    a2[:], pt, mybir.ActivationFunctionType.Sign,
                                    bias=nlo_ap)
                                nc.vector.tensor_tensor(
                                    y[:], a1[:], a2[:], mybir.AluOpType.add)
                            else:
                                b = outp.tile([C, NTv], dt.float32, tag="b")
                                nc.vector.tensor_scalar(
                                    b[:], pt, lo_ap, None, mybir.AluOpType.is_lt)
                                nc.vector.scalar_tensor_tensor(
                                    y[:], pt, hi_ap, b[:],
                                    mybir.AluOpType.is_gt, mybir.AluOpType.subtract)
                            out_eng = nc.scalar if cfg["act_out"] else nc.sync
                            j_i = grp.index(j)
                            if cfg["dmaless"]:
                                out_eng.dma_start(out_d[n, oc, j][:, :8], y[:, :8])
                            else:
                                out_eng.dma_start(out_d[n, oc, j], y[:])

        if repeat == 1:
            body()
        else:
            with tc.For_i(0, repeat, 1):
                body()

    nc.compile()
    return nc, (np.float16 if not cfg["f32r"] else np.float32)


def _parse_v3(mode):
    """v3: group-fused epilogue. 'v3[+sgn|+dve][+a][+t1][+dl][+g<k>]'
    default epilogue: mix (group-parity alternating ACT/DVE)."""
    parts = mode.split("+")
    assert parts[0] == "v3"
    flags = set(parts[1:])
    cfg = {
        "epi": ("sgn" if "sgn" in flags else
                "dve" if "dve" in flags else "mix"),
        "act_out": "a" in flags,
        "t1": "t1" in flags,
        "dl": "dl" in flags,
        "epb": "epb" in flags,   # bisect: per-block 2D epilogue ops
        "fb": "fb" in flags,     # full-bank 2D fused epilogue (512-col banks)
        "u2": "u2" in flags,     # unroll timing loop 2x for x double-buffering
        "G": 4,
    }
    for f in flags:
        if f.startswith("g") and f[1:].isdigit():
            cfg["G"] = int(f[1:])
    return cfg


NTV = RB * W  # 448 valid columns per row-block


def _build_v3(mode, repeat=1):
    import concourse.tile as tile
    from concourse import mybir, bacc
    from contextlib import ExitStack

    dt = mybir.dt
    nc = bacc.Bacc()
    cfg = _parse_v3(mode)
    G = cfg["G"]
    mm_dt = dt.float16
    out_dt = dt.bfloat16
    Sign = mybir.ActivationFunctionType.Sign

    CW = 512 if cfg["fb"] else NTV       # stored columns per row-block
    xh_d = nc.declare_dram_parameter("xh", [C, XCOLS_PAD], mm_dt, isOutput=False)
    sw_d = nc.declare_dram_parameter("sw", [C, 9 * O], mm_dt, isOutput=False)
    thr_d = nc.declare_dram_parameter("thr", [C, 8], dt.float32, isOutput=False)
    out_d = nc.declare_dram_parameter("out", [NPC, 2, C, NBLK * CW],
                                      out_dt, isOutput=True)

    taps = TAPS[:1] if cfg["t1"] else TAPS
    nmm = len(taps)
    groups = [list(range(g, min(g + G, NBLK))) for g in range(0, NBLK, G)]
    NGRP = len(groups)

    with tile.TileContext(nc) as tc, ExitStack() as ctx:
        inp = ctx.enter_context(tc.tile_pool(name="inp", bufs=2))
        outp = ctx.enter_context(tc.tile_pool(name="outp", bufs=4))
        ypool = ctx.enter_context(tc.tile_pool(name="ypool", bufs=8))
        psum = ctx.enter_context(tc.tile_pool(name="psum", bufs=2, space="PSUM"))

        def body():
            t_w = inp.tile([C, 9 * O], mm_dt, tag="w")
            nc.sync.dma_start(t_w[:], sw_d[:])
            t_thr = inp.tile([C, 8], dt.float32, tag="thr")
            nc.sync.dma_start(t_thr[:], thr_d[:])

            t_xh = inp.tile([C, XCOLS_PAD], mm_dt, tag="xh")
            bounds = [0, IMG, 2 * IMG, 3 * IMG, XCOLS_PAD]
            for i in range(4):
                lo, hi = bounds[i], bounds[i + 1]
                nc.sync.dma_start(t_xh[:, lo:hi], xh_d[:, lo:hi])

            for n in range(NPC):
                for oc in range(2):
                    lo_ap = t_thr[:, 2 * oc + 1:2 * oc + 2]
                    nhi_ap = t_thr[:, 4 + 2 * oc:4 + 2 * oc + 1]
                    nlo_ap = t_thr[:, 4 + 2 * oc + 1:4 + 2 * oc + 2]
                    for gi, grp in enumerate(groups):
                        g = len(grp)
                        ptg = psum.tile([C, G * 512], dt.float32, tag="ptg")
                        # tap-major MMs into per-bank 448-col slices
                        for t, (kh, kw) in enumerate(taps):
                            wt = t_w[:, t * O + oc * C: t * O + oc * C + C]
                            for bi, j in enumerate(grp):
                                base = n * IMG + (j * RB + kh) * WP + kw
                                rhs = (t_xh[:, base:base + NT]
                                       .rearrange("p (r w) -> p r w", w=WP)
                                       [:, :, :W])
                                nc.tensor.matmul(
                                    ptg[:, bi * 512: bi * 512 + NTV], wt, rhs,
                                    start=(t == 0), stop=(t == nmm - 1))
                        # fused epilogue over the whole group
                        gw = g * NTV
                        par = (gi + oc * NGRP + n * 2 * NGRP) % 2
                        epi = cfg["epi"]
                        use_sgn = (epi == "sgn") or (epi == "mix" and par == 0)
                        if cfg["fb"]:
                            # full-bank 2D fused ops incl. junk columns
                            gw5 = g * 512
                            ap_pt = ptg[:, :gw5]
                            y = ypool.tile([C, G * 512], out_dt, tag="y",
                                           name="y")[:, :gw5]
                            a1 = outp.tile([C, G * 512], out_dt, tag="a1",
                                           name="a1")[:, :gw5]
                            nc.scalar.activation(a1, ap_pt, Sign, bias=nhi_ap)
                            if use_sgn:
                                a2 = outp.tile([C, G * 512], out_dt, tag="a2",
                                               name="a2")[:, :gw5]
                                nc.scalar.activation(a2, ap_pt, Sign,
                                                     bias=nlo_ap)
                                nc.vector.tensor_tensor(
                                    y, a1, a2, mybir.AluOpType.add)
                            else:
                                b = outp.tile([C, G * 512], out_dt, tag="b",
                                              name="b")[:, :gw5]
                                nc.vector.tensor_scalar(
                                    b, ap_pt, lo_ap, None,
                                    mybir.AluOpType.is_lt)
                                nc.vector.scalar_tensor_tensor(
                                    y, b, -2.0, a1,
                                    mybir.AluOpType.mult, mybir.AluOpType.add)
                            out_eng = nc.scalar if cfg["act_out"] else nc.sync
                            j0 = grp[0]
                            if cfg["dl"]:
                                out_eng.dma_start(
                                    out_d[n, oc, :, j0 * 512:j0 * 512 + 8],
                                    y[:, :8])
                            else:
                                out_eng.dma_start(
                                    out_d[n, oc, :, j0 * 512:(j0 + g) * 512], y)
                            continue
                        y = ypool.tile([C, G * NTV], out_dt, tag="y", name="y")[:, :gw]
                        a1 = outp.tile([C, G * NTV], out_dt, tag="a1", name="a1")[:, :gw]
                        if cfg["epb"]:
                            # per-block 2D ops into the fused y tile
                            for bi in range(g):
                                pt2 = ptg[:, bi * 512: bi * 512 + NTV]
                                ya = y[:, bi * NTV:(bi + 1) * NTV]
                                aa = a1[:, bi * NTV:(bi + 1) * NTV]
                                nc.scalar.activation(aa, pt2, Sign, bias=nhi_ap)
                                if use_sgn:
                                    a2 = outp.tile([C, G * NTV], out_dt,
                                                   tag="a2", name="a2")
                                    a2b = a2[:, bi * NTV:(bi + 1) * NTV]
                                    nc.scalar.activation(a2b, pt2, Sign,
                                                         bias=nlo_ap)
                                    nc.vector.tensor_tensor(
                                        ya, aa, a2b, mybir.AluOpType.add)
                                else:
                                    b = outp.tile([C, G * NTV], out_dt,
                                                  tag="b", name="b")
                                    bb = b[:, bi * NTV:(bi + 1) * NTV]
                                    nc.vector.tensor_scalar(
                                        bb, pt2, lo_ap, None,
                                        mybir.AluOpType.is_lt)
                                    nc.vector.scalar_tensor_tensor(
                                        ya, bb, -2.0, aa,
                                        mybir.AluOpType.mult,
                                        mybir.AluOpType.add)
                        else:
                            ap_pt = (ptg[:, :g * 512]
                                     .rearrange("p (g k) -> p g k", k=512)
                                     [:, :, :NTV])
                            ap_a1 = a1.rearrange("p (g k) -> p g k", k=NTV)
                            nc.scalar.activation(ap_a1, ap_pt, Sign, bias=nhi_ap)
                            if use_sgn:
                                a2 = outp.tile([C, G * NTV], out_dt, tag="a2", name="a2")[:, :gw]
                                ap_a2 = a2.rearrange("p (g k) -> p g k", k=NTV)
                                nc.scalar.activation(ap_a2, ap_pt, Sign, bias=nlo_ap)
                                nc.vector.tensor_tensor(
                                    y, a1, a2, mybir.AluOpType.add)
                            else:
                                b = outp.tile([C, G * NTV], out_dt, tag="b", name="b")[:, :gw]
                                ap_b = b.rearrange("p (g k) -> p g k", k=NTV)
                                nc.vector.tensor_scalar(
                                    ap_b, ap_pt, lo_ap, None, mybir.AluOpType.is_lt)
                                nc.vector.scalar_tensor_tensor(
                                    y, b, -2.0, a1,
                                    mybir.AluOpType.mult, mybir.AluOpType.add)
                        out_eng = nc.scalar if cfg["act_out"] else nc.sync
                        j0 = grp[0]
                        if cfg["dl"]:
                            out_eng.dma_start(
                                out_d[n, oc, :, j0 * NTV:j0 * NTV + 8], y[:, :8])
                        else:
                            out_eng.dma_start(
                                out_d[n, oc, :, j0 * NTV:(j0 + g) * NTV], y)

        if repeat == 1:
            body()
        elif cfg["u2"] and repeat % 2 == 0:
            # unroll 2x so tile-pool rotation double-buffers across
            # loop iterations (addresses are fixed inside a HW loop)
            with tc.For_i(0, repeat // 2, 1):
                body()
                body()
        else:
            with tc.For_i(0, repeat, 1):
                body()

    nc.compile()
    return nc, np.float16


def _build(mode, repeat=1):
    if mode.split("+")[0] == "v2":
        return _build_v2(mode, repeat)
    if mode.split("+")[0] == "v3":
        return _build_v3(mode, repeat)
    import concourse.tile as tile
    from concourse import mybir, bacc
    from contextlib import ExitStack

    dt = mybir.dt
    nc = bacc.Bacc()

    parts = mode.split("+")
    base, flags = parts[0], set(parts[1:])
    jpad = "j" in flags
    ydt_bf = "h" in flags       # store ternary output as bf16 (host converts)
    dmaless = "dl" in flags     # timing probe: ~zero output DMA volume
    act_out = "a" in flags      # issue output stores on the ACT HWDGE ring
    obufs = 16 if "o16" in flags else 6
    ysplit = "ys" in flags      # dedicated deep pool for DMA-held y tiles
    ne1 = "ne" in flags         # probe: only 1 DVE epilogue op
    ne2 = "ne2" in flags        # probe: no DVE epilogue at all

    if base == "f16p":
        mm_dt, np_mm = dt.float16, np.float16
    elif base == "f16s":
        mm_dt, np_mm = dt.float16, np.float16
    elif base == "bf16s":
        mm_dt, np_mm = dt.bfloat16, np.float32
    elif base == "f32r":
        mm_dt, np_mm = dt.float32r, np.float32
    elif base == "f32":
        mm_dt, np_mm = dt.float32, np.float32
    else:
        raise ValueError(mode)
    pair = base == "f16p"

    xh_d = nc.declare_dram_parameter("xh", [C, XCOLS_PAD], mm_dt, isOutput=False)
    xl_d = (nc.declare_dram_parameter("xl", [C, XCOLS_PAD], mm_dt, isOutput=False)
            if pair else None)
    sw_d = nc.declare_dram_parameter("sw", [C, 9 * O], mm_dt, isOutput=False)
    thr_d = nc.declare_dram_parameter("thr", [C, 4], dt.float32, isOutput=False)
    out_dt = dt.bfloat16 if ydt_bf else dt.float32
    if jpad:
        out_d = nc.declare_dram_parameter("out", [NPC, 2, NBLK, C, NT],
                                          out_dt, isOutput=True)
    else:
        out_d = nc.declare_dram_parameter("out", [2, C, NPC, H, W],
                                          out_dt, isOutput=True)

    with tile.TileContext(nc) as tc, ExitStack() as ctx:
        inp = ctx.enter_context(tc.tile_pool(name="inp", bufs=2))
        outp = ctx.enter_context(tc.tile_pool(name="outp", bufs=4 if ysplit else obufs))
        ypool = (ctx.enter_context(tc.tile_pool(name="ypool", bufs=24))
                 if ysplit else outp)
        psum = ctx.enter_context(tc.tile_pool(name="psum", bufs=8, space="PSUM"))

        def body():
            t_w = inp.tile([C, 9 * O], mm_dt, tag="w")
            nc.sync.dma_start(t_w[:], sw_d[:])
            t_thr = inp.tile([C, 4], dt.float32, tag="thr")
            nc.sync.dma_start(t_thr[:], thr_d[:])

            t_xh = inp.tile([C, XCOLS_PAD], mm_dt, tag="xh")
            t_xl = (inp.tile([C, XCOLS_PAD], mm_dt, tag="xl", name="t_xl")
                    if pair else None)
            # chunked x loads (per image) so compute starts after chunk 0
            bounds = [0, IMG, 2 * IMG, 3 * IMG, XCOLS_PAD]
            for i in range(4):
                lo, hi = bounds[i], bounds[i + 1]
                nc.sync.dma_start(t_xh[:, lo:hi], xh_d[:, lo:hi])
                if pair:
                    nc.sync.dma_start(t_xl[:, lo:hi], xl_d[:, lo:hi])

            nmm = 18 if pair else 9
            for n in range(NPC):
                for oc in range(2):
                    hi_ap = t_thr[:, 2 * oc:2 * oc + 1]
                    lo_ap = t_thr[:, 2 * oc + 1:2 * oc + 2]
                    for j in range(NBLK):
                        h0 = j * RB
                        pt = psum.tile([C, 512], dt.float32, tag="pt")
                        pt = pt[:, :NT]
                        k = 0
                        for t, (kh, kw) in enumerate(TAPS):
                            base_off = n * IMG + (h0 + kh) * WP + kw
                            wt = t_w[:, t * O + oc * C: t * O + oc * C + C]
                            nc.tensor.matmul(pt, wt, t_xh[:, base_off:base_off + NT],
                                             start=(k == 0), stop=(k == nmm - 1))
                            k += 1
                            if pair:
                                nc.tensor.matmul(pt, wt, t_xl[:, base_off:base_off + NT],
                                                 start=False, stop=(k == nmm - 1))
                                k += 1
                        # ternary epilogue: y = (raw > hi) - (raw < lo)
                        y = ypool.tile([C, NT], out_dt, tag="y")
                        if ne2:
                            # probe: PSUM->SBUF copy on ACT, no DVE at all
                            nc.scalar.copy(y[:], pt)
                        elif ne1:
                            nc.vector.tensor_scalar(y[:], pt, hi_ap, None,
                                                    mybir.AluOpType.is_gt)
                        else:
                            b = outp.tile([C, NT], dt.float32, tag="b")
                            nc.vector.tensor_scalar(b[:], pt, lo_ap, None,
                                                    mybir.AluOpType.is_lt)
                            nc.vector.scalar_tensor_tensor(
                                y[:], pt, hi_ap, b[:],
                                mybir.AluOpType.is_gt, mybir.AluOpType.subtract)
                        if jpad:
                            out_eng = nc.scalar if act_out else nc.sync
                            if dmaless:
                                out_eng.dma_start(out_d[n, oc, j][:, :8], y[:, :8])
                            else:
                                out_eng.dma_start(out_d[n, oc, j], y[:])
                        else:
                            y_r = y[:].rearrange("p (r w) -> p r w", w=WP)[:, :, :W]
                            nc.sync.dma_start(out_d[oc, :, n, h0:h0 + RB, :], y_r)

        if repeat == 1:
            body()
        else:
            with tc.For_i(0, repeat, 1):
                body()

    nc.compile()
    return nc, np_mm


def _host_prep(x, weight, bias):
    scale = np.clip(np.mean(np.abs(weight), axis=(1, 2, 3)), 1e-8, None)  # [O]
    sw = np.sign(weight)                                                  # [O,C,3,3]
    hi = ((0.5 - bias.astype(np.float64)) / scale.astype(np.float64)).astype(np.float32)
    lo = ((-0.5 - bias.astype(np.float64)) / scale.astype(np.float64)).astype(np.float32)
    thr = np.stack([hi[:C], lo[:C], hi[C:], lo[C:]], axis=1).astype(np.float32)
    # lhsT layout: sw[c, t*O + o]
    swt = np.ascontiguousarray(sw.transpose(1, 2, 3, 0).reshape(C, 9 * O))
    # pad x to 58x58 and lay out [C, n*3364 + hp*58 + wp]
    xp = np.zeros((N, C, HP, WP), dtype=np.float32)
    xp[:, :, 1:-1, 1:-1] = x
    xp = xp.transpose(1, 0, 2, 3).reshape(C, N * IMG)
    return thr, swt, xp


def _make_in_maps(mode, thr, swt, xp):
    base = mode.split("+")[0]
    if base == "v3":
        thr8 = np.concatenate([thr, -thr], axis=1).astype(np.float32)
        in_maps = []
        for c in range(NCORES):
            xc = np.zeros((C, XCOLS_PAD), dtype=np.float32)
            xc[:, :XCOLS] = xp[:, c * XCOLS:(c + 1) * XCOLS]
            in_maps.append({"thr": thr8, "sw": swt.astype(np.float16),
                            "xh": xc.astype(np.float16)})
        return in_maps
    if base == "v2":
        cfg = _parse_v2(mode)
        thr8 = np.concatenate([thr, -thr], axis=1).astype(np.float32)  # [C,8]
        np_mm = np.float32 if cfg["f32r"] else np.float16
        in_maps = []
        for c in range(NCORES):
            xc = np.zeros((C, XCOLS_PAD), dtype=np.float32)
            xc[:, :XCOLS] = xp[:, c * XCOLS:(c + 1) * XCOLS]
            m = {"thr": thr8, "sw": swt.astype(np_mm)}
            if cfg["pair"]:
                xh = xc.astype(np.float16)
                m["xh"] = xh
                m["xl"] = (xc - xh.astype(np.float32)).astype(np.float16)
            else:
                m["xh"] = xc.astype(np_mm)
            in_maps.append(m)
        return in_maps
    pair = base == "f16p"
    in_maps = []
    for c in range(NCORES):
        xc = np.zeros((C, XCOLS_PAD), dtype=np.float32)
        xc[:, :XCOLS] = xp[:, c * XCOLS:(c + 1) * XCOLS]
        m = {"thr": thr}
        if pair:
            xh = xc.astype(np.float16)
            m["xh"] = xh
            m["xl"] = (xc - xh.astype(np.float32)).astype(np.float16)
            m["sw"] = swt.astype(np.float16)
        elif base == "f16s":
            m["xh"] = xc.astype(np.float16)
            m["sw"] = swt.astype(np.float16)
        elif base == "bf16s":
            import ml_dtypes
            m["xh"] = xc.astype(ml_dtypes.bfloat16)
            m["sw"] = swt.astype(ml_dtypes.bfloat16)
        else:
            m["xh"] = xc
            m["sw"] = swt.copy()
        in_maps.append(m)
    return in_maps


def kernel(x, weight, bias):
    from concourse.bass_utils import run_bass_kernel_spmd

    x = np.asarray(x, dtype=np.float32)
    weight = np.asarray(weight, dtype=np.float32)
    bias = np.asarray(bias, dtype=np.float32)

    thr, swt, xp = _host_prep(x, weight, bias)

    mode = MODE
    if mode not in _prog_cache:
        _prog_cache[mode] = _build(mode)
    nc, _ = _prog_cache[mode]

    in_maps = _make_in_maps(mode, thr, swt, xp)
    res = run_bass_kernel_spmd(nc, in_maps, list(range(NCORES)))

    # ---- gather per-core outputs -> [N, O, H, W] fp32 ----
    out = np.empty((N, O, H, W), dtype=np.float32)
    if mode.split("+")[0] == "v3":
        cfg = _parse_v3(mode)
        G = cfg["G"]
        groups = [list(range(g, min(g + G, NBLK))) for g in range(0, NBLK, G)]
        NGRP = len(groups)
        CW = 512 if cfg["fb"] else NTV
        # per-(n,oc) column parity vector [NBLK*CW] for mix decode
        for c in range(NCORES):
            v = np.asarray(res.results[c]["out"]).astype(np.float32, copy=False)
            v = v.reshape(NPC, 2, C, NBLK * CW)
            if cfg["epi"] == "sgn":
                v = v * 0.5
            else:  # mix: y = v*0.5 + par*0.5
                par = np.zeros((NPC, 2, 1, NBLK * CW), dtype=np.float32)
                for n_i in range(NPC):
                    for oc_i in range(2):
                        cols = []
                        for gi, grp in enumerate(groups):
                            p = (gi + oc_i * NGRP + n_i * 2 * NGRP) % 2
                            cols.append(np.full(len(grp) * CW, p, np.float32))
                        par[n_i, oc_i, 0] = np.concatenate(cols)
                if cfg["epi"] == "dve":
                    par[:] = 1.0
                v = v * 0.5 + par * 0.5
            v = v.reshape(NPC, 2, C, NBLK, CW)[:, :, :, :, :NTV]
            v = v.reshape(NPC, 2, C, NBLK, RB, W).reshape(NPC, O, H, W)
            out[c * NPC:(c + 1) * NPC] = v
        return out
    if mode.split("+")[0] == "v2":
        cfg = _parse_v2(mode)
        wv = W if cfg["v448"] else WP
        if cfg["mix"]:
            n_i, oc_i, j_i = np.meshgrid(np.arange(NPC), np.arange(2),
                                         np.arange(NBLK), indexing="ij")
            par = ((j_i + oc_i * NBLK + n_i * 2 * NBLK) % 2).astype(np.float32)
            par = par[:, :, :, None, None, None]
        for c in range(NCORES):
            v = np.asarray(res.results[c]["out"]).astype(np.float32, copy=False)
            v = v.reshape(NPC, 2, NBLK, C, RB, wv)
            if cfg["mix"]:
                v = v * 0.5 + par * 0.5
            elif cfg["sgn"]:
                v = v * 0.5
            v = v[:, :, :, :, :, :W]
            v = v.transpose(0, 1, 3, 2, 4, 5).reshape(NPC, O, H, W)
            out[c * NPC:(c + 1) * NPC] = v
        return out
    for c in range(NCORES):
        oc_out = res.results[c]["out"]
        if "+j" in mode:
            # [NPC, 2, NBLK, C, NT]: rows of 58, valid w < 56
            v = np.asarray(oc_out).astype(np.float32, copy=False)
            v = v.reshape(NPC, 2, NBLK, C, RB, WP)[:, :, :, :, :, :W]
            v = v.transpose(0, 1, 3, 2, 4, 5).reshape(NPC, O, H, W)
            out[c * NPC:(c + 1) * NPC] = v
        else:
            for oc in range(2):
                out[c * NPC:(c + 1) * NPC, oc * C:(oc + 1) * C] = \
                    oc_out[oc].transpose(1, 0, 2, 3)
    return out

